# revision 2
# baseline (speedup 1.0000x reference)
"""Viterbi CRF decode on 8 Trainium2 NeuronCores.

Strategy: data-parallel over batch (32 sequences/core). The device kernel runs
the forward max-plus DP (alpha recurrence, the dominant compute) and streams the
full alpha history back to HBM. The host then does the O(L*B*T) backtrack over
that history (0.03% of the FLOPs) plus the sequence-length freeze handling.

Exactness: the device computes alpha_t[j] = max_i(fp32(alpha_{t-1}[i] +
trans[i,j])) + pot_t[j] with the same fp32 rounding as the jax reference, so the
backtrack argmax decisions (first-index tie-break) match bitwise.

Device layout per step (128 partitions = 4 j-quadrants x 32 sequences):
  vt[(q,b), (jb,i)] = alpha[b,i] + trans[i, 16q+jb]   (broadcast add, 1024/partition)
  m4[(q,b), jb]     = max_i vt                        (free-dim reduce)
  alpha'[b, 16q+jb] = m4[(q,b), jb] + pot             (4 collapse copies + add)
"""

import numpy as np

B, L, T = 256, 1024, 64
NCORES = 8
BC = B // NCORES  # 32 sequences per core
CH = 128          # potentials chunk (steps per DMA)

_cache = {}


def _build_program():
    if "nc" in _cache:
        return _cache["nc"]
    import concourse.bacc as bacc
    import concourse.mybir as mybir
    from concourse.tile import TileContext

    f32 = mybir.dt.float32
    AX = mybir.AxisListType
    OP = mybir.AluOpType

    nc = bacc.Bacc("TRN2", target_bir_lowering=False, debug=False)
    pots_in = nc.dram_tensor("pots", [BC, L, T], f32, kind="ExternalInput").ap()
    tsp_in = nc.dram_tensor("tspread", [128, 16, T], f32, kind="ExternalInput").ap()
    hist_out = nc.dram_tensor("ahist", [BC, L, T], f32, kind="ExternalOutput").ap()

    JBD = 12  # jb 0:12 added on DVE, 12:16 on Pool (DVE ~1.07, Pool ~3.0 ns/elem)

    with TileContext(nc) as tc:
        with tc.tile_pool(name="const", bufs=1) as cpool, \
             tc.tile_pool(name="pstream", bufs=2) as ppool, \
             tc.tile_pool(name="work", bufs=3) as wpool, \
             tc.tile_pool(name="big", bufs=1) as bpool:
            tsp = cpool.tile([128, 16, T], f32)
            nc.gpsimd.dma_start(out=tsp[:], in_=tsp_in[:])
            hist = bpool.tile([128, 256, T], f32)   # alpha history, 64KB/partition
            arep = cpool.tile([128, T], f32)

            nchunks = L // CH
            for c in range(nchunks):
                pc = ppool.tile([BC, CH, T], f32, tag="pots")
                nc.gpsimd.dma_start(out=pc[:], in_=pots_in[:, c * CH:(c + 1) * CH, :])

                if c == 0:
                    nc.vector.tensor_copy(arep[0:BC, :], pc[:, 0, :])
                    nc.gpsimd.tensor_copy(hist[0:BC, 0, :], arep[0:BC, :])
                    nc.vector.tensor_copy(arep[BC:2 * BC, :], arep[0:BC, :])
                    nc.vector.tensor_copy(arep[2 * BC:4 * BC, :], arep[0:2 * BC, :])

                t0 = max(c * CH, 1)
                for t in range(t0, (c + 1) * CH):
                    tg, tl = t >> 8, t & 255
                    s = t - c * CH
                    # vt[p, jb, i] = alpha[p%32, i] + trans[i, 16*(p//32)+jb]
                    vt = wpool.tile([128, 16, T], f32, tag="vt")
                    nc.vector.tensor_add(
                        vt[:, 0:JBD, :],
                        arep[:].unsqueeze(1).broadcast_to([128, JBD, T]),
                        tsp[:, 0:JBD, :],
                    )
                    nc.gpsimd.tensor_add(
                        vt[:, JBD:16, :],
                        arep[:].unsqueeze(1).broadcast_to([128, 16 - JBD, T]),
                        tsp[:, JBD:16, :],
                    )
                    m4 = wpool.tile([128, 16], f32, tag="m4")
                    nc.vector.tensor_reduce(m4[:], vt[:], axis=AX.X, op=OP.max)
                    ab = wpool.tile([BC, T], f32, tag="ab")
                    nc.vector.tensor_copy(ab[:, 0:16], m4[0:BC, :])
                    nc.gpsimd.tensor_copy(ab[:, 16:32], m4[BC:2 * BC, :])
                    nc.vector.tensor_copy(ab[:, 32:48], m4[2 * BC:3 * BC, :])
                    nc.gpsimd.tensor_copy(ab[:, 48:64], m4[3 * BC:4 * BC, :])
                    nc.vector.tensor_add(arep[0:BC, :], ab[:], pc[:, s, :])
                    nc.scalar.copy(hist[BC * tg:BC * (tg + 1), tl, :], arep[0:BC, :])
                    nc.vector.tensor_copy(arep[BC:2 * BC, :], arep[0:BC, :])
                    nc.gpsimd.tensor_copy(arep[2 * BC:3 * BC, :], arep[0:BC, :])
                    nc.vector.tensor_copy(arep[3 * BC:4 * BC, :], arep[0:BC, :])

            for tg in range(4):
                nc.gpsimd.dma_start(
                    out=hist_out[:, 256 * tg:256 * (tg + 1), :],
                    in_=hist[BC * tg:BC * (tg + 1), :, :],
                )

    nc.compile()
    _cache["nc"] = nc
    return nc


def _make_tspread(trans):
    # tsp[32q + b, jb, i] = trans[i, 16q + jb]
    tt = np.ascontiguousarray(trans.T).reshape(4, 16, T)  # [q, jb, i]
    return np.repeat(tt[:, None, :, :], BC, axis=1).reshape(128, 16, T).astype(np.float32)


def _make_in_maps(potentials, trans):
    tsp = _make_tspread(trans)
    return [
        {"pots": potentials[c * BC:(c + 1) * BC], "tspread": tsp}
        for c in range(NCORES)
    ]


def kernel(potentials, lengths, transition_params):
    from concourse.bass_utils import run_bass_kernel_spmd

    potentials = np.ascontiguousarray(np.asarray(potentials, dtype=np.float32))
    lengths = np.asarray(lengths, dtype=np.int32)
    trans = np.ascontiguousarray(np.asarray(transition_params, dtype=np.float32))

    nc = _build_program()
    tsp = _make_tspread(trans)
    in_maps = [
        {"pots": potentials[c * BC:(c + 1) * BC], "tspread": tsp}
        for c in range(NCORES)
    ]
    res = run_bass_kernel_spmd(nc, in_maps, core_ids=list(range(NCORES)))
    ah = np.concatenate([res.results[c]["ahist"] for c in range(NCORES)], axis=0)

    # Host backtrack over the device-computed alpha history.
    tags = np.zeros((B, L), dtype=np.int64)
    last = ah[np.arange(B), lengths - 1, :].argmax(axis=1)
    tags[:, L - 1] = last
    lm1 = lengths - 1
    for t in range(L - 2, -1, -1):
        nxt = tags[:, t + 1]
        cand = ah[:, t, :] + trans[:, nxt].T
        tags[:, t] = np.where(t >= lm1, last, cand.argmax(axis=1))
    return tags.astype(np.int32)



# revision 4
# speedup vs baseline: 1.0217x; 1.0217x over previous
"""Viterbi CRF decode on 8 Trainium2 NeuronCores — v2 (custom DVE segmax).

Data-parallel over batch (32 seqs/core). Per step, the max-plus inner loop
  m[b,j] = max_i fp32(alpha[b,i] + trans[i,j])
runs as ONE custom DVE instruction (VITERBI_SEGMAX): a fused
(Src0 + Src1) running-max scan with a page-boundary reset patched into the
uop program, over layout [128 partitions=(q,b), 16 pages=jb, 64=i] where
j = 16q + jb. Segment-end elements X[:, :, 63] are the 16 maxes per
partition, bitwise equal to the jax reference (single fp32 add, exact max).

The rest of the step stays on the DVE to avoid cross-engine sem latency:
4 fused "pot-add + collapse" TTs write alpha[b, :] directly into arep[0:32]
(cross-partition-offset outputs), then 2 doubling copies rebroadcast to all
128 partitions. The alpha history write goes to the Scalar engine
(off-critical-path). Host backtrack over the history as in v1.
"""

import numpy as np

B, L, T = 256, 1024, 64
NCORES = 8
BC = B // NCORES   # 32 sequences per core
CH = 128           # potq chunk (steps per input DMA)

_cache = {}


def _register_segmax():
    """Register the VITERBI_SEGMAX custom DVE op (idempotent).

    out[p,s,n] = running max within page s of fp32(in0[p,s,n] + in1[p,s,n]).
    The stock scan() has no page reset; we patch the lowering so the
    SUB_DIM_DONE step state re-seeds the scan stage from the current
    element's (Src0+Src1) value instead of folding the carried max.
    """
    import contextlib

    from concourse import dve_spec
    from concourse.dve_spec import AluOp, Spec, Src0, Src1, lower, scan
    from concourse.dve_uop import DveOpSpec
    from concourse.dve_ops import (
        _CUSTOM_DVE_ROW_BASE,
        _SUB_OPCODE_FOR_NAME,
        CUSTOM_DVE_SPECS,
        OPS,
        DveOp,
    )

    name = "VITERBI_SEGMAX"
    for op in OPS:
        if op.name == name:
            return op

    def _reference(in0, in1, **_kw):
        return np.maximum.accumulate(
            in0.astype(np.float32) + in1.astype(np.float32), axis=-1
        )

    spec = Spec(body=scan(AluOp.MAX, Src0 + Src1), reference=_reference)

    @contextlib.contextmanager
    def _page_reset_patch():
        orig = dve_spec._scan_overrides

        def patched(scans, node_stage):
            seed, step = orig(scans, node_stage)
            for s in scans:
                if s._subdim_step is None:
                    step[node_stage[s]] = dve_spec._Stage(AluOp.BYPASS, s.expr)
            return seed, step

        dve_spec._scan_overrides = patched
        try:
            yield
        finally:
            dve_spec._scan_overrides = orig

    row = _CUSTOM_DVE_ROW_BASE + len(OPS)
    _SUB_OPCODE_FOR_NAME[name] = row
    with _page_reset_patch():
        shas = {}
        for ver in ("v3", "v4"):
            uops = lower(spec, ver=ver)
            shas[ver] = DveOpSpec(
                name=name, opcode=row, uops=uops, rd1_en=True
            ).sha(ver)
        op = DveOp(name, spec, subdim=True, uops_sha=shas)
        OPS.append(op)
        CUSTOM_DVE_SPECS[name] = spec
        op.compile("v3")
        op.compile("v4")
    return op


def _build_program():
    if "nc" in _cache:
        return _cache["nc"]
    import concourse.bacc as bacc
    import concourse.mybir as mybir
    from concourse.tile import TileContext

    segmax_op = _register_segmax()

    f32 = mybir.dt.float32

    nc = bacc.Bacc("TRN2", target_bir_lowering=False, debug=False)
    potq_in = nc.dram_tensor("potq", [128, L, 16], f32, kind="ExternalInput").ap()
    tsp_in = nc.dram_tensor("tspread", [128, 16, T], f32, kind="ExternalInput").ap()
    hist_out = nc.dram_tensor("ahist", [BC, L, T], f32, kind="ExternalOutput").ap()

    with TileContext(nc) as tc:
        with tc.tile_pool(name="const", bufs=1) as cpool, \
             tc.tile_pool(name="pstream", bufs=2) as ppool, \
             tc.tile_pool(name="work", bufs=2) as wpool, \
             tc.tile_pool(name="big", bufs=1) as bpool:
            tsp = cpool.tile([128, 16, T], f32)
            nc.gpsimd.dma_start(out=tsp[:], in_=tsp_in[:])
            arep = cpool.tile([128, T], f32)
            hist = bpool.tile([128, 256, T], f32)   # 64KB/partition

            arep_bc = arep[:].unsqueeze(1).broadcast_to([128, 16, T])

            CH0 = 16
            bounds = [0, CH0] + [CH0 + CH * k for k in range(1, (L - CH0) // CH + 1)]
            if bounds[-1] != L:
                bounds.append(L)
            for c in range(len(bounds) - 1):
                lo, hi = bounds[c], bounds[c + 1]
                pq = ppool.tile([128, CH, 16], f32, tag="potq")
                nc.gpsimd.dma_start(out=pq[:, 0:hi - lo, :], in_=potq_in[:, lo:hi, :])

                for s in range(hi - lo):
                    t = lo + s
                    tg, tl = t >> 8, t & 255
                    if t == 0:
                        for q in range(4):
                            nc.vector.tensor_copy(
                                arep[0:BC, 16 * q:16 * (q + 1)],
                                pq[BC * q:BC * (q + 1), 0, :],
                            )
                    else:
                        x = wpool.tile([128, 16, T], f32, tag="x")
                        nc.vector._custom_dve(
                            segmax_op, out=x[:], in0=tsp[:], in1=arep_bc
                        )
                        # fused pot-add + collapse: alpha'[b, 16q+jb]
                        for q in range(4):
                            nc.vector.tensor_add(
                                arep[0:BC, 16 * q:16 * (q + 1)],
                                x[BC * q:BC * (q + 1), :, T - 1],
                                pq[BC * q:BC * (q + 1), s, :],
                            )
                    # alpha history (off critical path, Scalar engine)
                    nc.scalar.copy(hist[BC * tg:BC * (tg + 1), tl, :], arep[0:BC, :])
                    # rebroadcast alpha to the other 3 quadrant groups
                    nc.vector.tensor_copy(arep[BC:2 * BC, :], arep[0:BC, :])
                    nc.vector.tensor_copy(arep[2 * BC:4 * BC, :], arep[0:2 * BC, :])

                    if tg < 3 and (t + 1) % 256 == 0:
                        nc.gpsimd.dma_start(
                            out=hist_out[:, 256 * tg:256 * (tg + 1), :],
                            in_=hist[BC * tg:BC * (tg + 1), :, :],
                        )
                    elif tg == 3 and (t + 1) % 64 == 0:
                        h0 = ((t + 1) // 64 - 1) * 64
                        nc.gpsimd.dma_start(
                            out=hist_out[:, h0:t + 1, :],
                            in_=hist[3 * BC:4 * BC, h0 - 768:t + 1 - 768, :],
                        )

    nc.compile()
    _cache["nc"] = nc
    return nc


def _make_tspread(trans):
    # tsp[32q + b, jb, i] = trans[i, 16q + jb]
    tt = np.ascontiguousarray(trans.T).reshape(4, 16, T)  # [q, jb, i]
    return np.repeat(tt[:, None, :, :], BC, axis=1).reshape(128, 16, T).astype(np.float32)


def _make_potq(pots_core):
    # potq[32q + b, t, jb] = pots_core[b, t, 16q + jb]
    return np.ascontiguousarray(
        pots_core.reshape(BC, L, 4, 16).transpose(2, 0, 1, 3).reshape(128, L, 16)
    )


def _make_in_maps(potentials, trans):
    tsp = _make_tspread(trans)
    return [
        {"potq": _make_potq(potentials[c * BC:(c + 1) * BC]), "tspread": tsp}
        for c in range(NCORES)
    ]


def kernel(potentials, lengths, transition_params):
    from concourse.bass_utils import run_bass_kernel_spmd

    potentials = np.ascontiguousarray(np.asarray(potentials, dtype=np.float32))
    lengths = np.asarray(lengths, dtype=np.int32)
    trans = np.ascontiguousarray(np.asarray(transition_params, dtype=np.float32))

    nc = _build_program()
    in_maps = _make_in_maps(potentials, trans)
    res = run_bass_kernel_spmd(nc, in_maps, core_ids=list(range(NCORES)))
    ah = np.concatenate([res.results[c]["ahist"] for c in range(NCORES)], axis=0)

    # Host backtrack over the device-computed alpha history.
    tags = np.zeros((B, L), dtype=np.int64)
    last = ah[np.arange(B), lengths - 1, :].argmax(axis=1)
    tags[:, L - 1] = last
    lm1 = lengths - 1
    for t in range(L - 2, -1, -1):
        nxt = tags[:, t + 1]
        cand = ah[:, t, :] + trans[:, nxt].T
        tags[:, t] = np.where(t >= lm1, last, cand.argmax(axis=1))
    return tags.astype(np.int32)


# revision 5
# speedup vs baseline: 1.0319x; 1.0100x over previous
"""Viterbi CRF decode on 8 Trainium2 NeuronCores — v2 (custom DVE segmax).

Data-parallel over batch (32 seqs/core). Per step, the max-plus inner loop
  m[b,j] = max_i fp32(alpha[b,i] + trans[i,j])
runs as ONE custom DVE instruction (VITERBI_SEGMAX): a fused
(Src0 + Src1) running-max scan with a page-boundary reset patched into the
uop program, over layout [128 partitions=(q,b), 16 pages=jb, 64=i] where
j = 16q + jb. Segment-end elements X[:, :, 63] are the 16 maxes per
partition, bitwise equal to the jax reference (single fp32 add, exact max).

The rest of the step stays on the DVE to avoid cross-engine sem latency:
4 fused "pot-add + collapse" TTs write alpha[b, :] directly into arep[0:32]
(cross-partition-offset outputs), then 2 doubling copies rebroadcast to all
128 partitions. The alpha history write goes to the Scalar engine
(off-critical-path). Host backtrack over the history as in v1.
"""

import numpy as np

B, L, T = 256, 1024, 64
NCORES = 8
BC = B // NCORES   # 32 sequences per core
CH = 128           # potq chunk (steps per input DMA)

_cache = {}


def _register_segmax():
    """Register the VITERBI_SEGMAX custom DVE op (idempotent).

    out[p,s,n] = running max within page s of fp32(in0[p,s,n] + in1[p,s,n]).
    The stock scan() has no page reset; we patch the lowering so the
    SUB_DIM_DONE step state re-seeds the scan stage from the current
    element's (Src0+Src1) value instead of folding the carried max.
    """
    import contextlib

    from concourse import dve_spec
    from concourse.dve_spec import AluOp, Spec, Src0, Src1, lower, scan
    from concourse.dve_uop import DveOpSpec
    from concourse.dve_ops import (
        _CUSTOM_DVE_ROW_BASE,
        _SUB_OPCODE_FOR_NAME,
        CUSTOM_DVE_SPECS,
        OPS,
        DveOp,
    )

    name = "VITERBI_SEGMAX"
    for op in OPS:
        if op.name == name:
            return op

    def _reference(in0, in1, **_kw):
        return np.maximum.accumulate(
            in0.astype(np.float32) + in1.astype(np.float32), axis=-1
        )

    spec = Spec(body=scan(AluOp.MAX, Src0 + Src1), reference=_reference)

    @contextlib.contextmanager
    def _page_reset_patch():
        orig = dve_spec._scan_overrides

        def patched(scans, node_stage):
            seed, step = orig(scans, node_stage)
            for s in scans:
                if s._subdim_step is None:
                    step[node_stage[s]] = dve_spec._Stage(AluOp.BYPASS, s.expr)
            return seed, step

        dve_spec._scan_overrides = patched
        try:
            yield
        finally:
            dve_spec._scan_overrides = orig

    row = _CUSTOM_DVE_ROW_BASE + len(OPS)
    _SUB_OPCODE_FOR_NAME[name] = row
    with _page_reset_patch():
        shas = {}
        for ver in ("v3", "v4"):
            uops = lower(spec, ver=ver)
            shas[ver] = DveOpSpec(
                name=name, opcode=row, uops=uops, rd1_en=True
            ).sha(ver)
        op = DveOp(name, spec, subdim=True, uops_sha=shas)
        OPS.append(op)
        CUSTOM_DVE_SPECS[name] = spec
        op.compile("v3")
        op.compile("v4")
    return op


def _build_program():
    if "nc" in _cache:
        return _cache["nc"]
    import concourse.bacc as bacc
    import concourse.mybir as mybir
    from concourse.tile import TileContext

    segmax_op = _register_segmax()

    f32 = mybir.dt.float32

    nc = bacc.Bacc("TRN2", target_bir_lowering=False, debug=False)
    potq_in = nc.dram_tensor("potq", [128, L, 16], f32, kind="ExternalInput").ap()
    tsp_in = nc.dram_tensor("tspread", [128, 16, T], f32, kind="ExternalInput").ap()
    hist_out = nc.dram_tensor("ahist", [BC, L, T], f32, kind="ExternalOutput").ap()

    with TileContext(nc) as tc:
        with tc.tile_pool(name="const", bufs=1) as cpool, \
             tc.tile_pool(name="pstream", bufs=2) as ppool, \
             tc.tile_pool(name="work", bufs=2) as wpool, \
             tc.tile_pool(name="big", bufs=1) as bpool:
            tsp = cpool.tile([128, 16, T], f32)
            nc.gpsimd.dma_start(out=tsp[:], in_=tsp_in[:])
            # two alpha slots (t%2): lets one ACT copy capture 2 steps of
            # history, halving the DVE<->ACT sem edges
            arep = cpool.tile([128, 2, T], f32)
            hist = bpool.tile([128, 256, T], f32)   # 64KB/partition

            arep_bc = [
                arep[:, 0, :].unsqueeze(1).broadcast_to([128, 16, T]),
                arep[:, 1, :].unsqueeze(1).broadcast_to([128, 16, T]),
            ]

            CH0 = 16
            bounds = [0, CH0] + [CH0 + CH * k for k in range(1, (L - CH0) // CH + 1)]
            if bounds[-1] != L:
                bounds.append(L)
            for c in range(len(bounds) - 1):
                lo, hi = bounds[c], bounds[c + 1]
                pq = ppool.tile([128, CH, 16], f32, tag="potq")
                nc.gpsimd.dma_start(out=pq[:, 0:hi - lo, :], in_=potq_in[:, lo:hi, :])

                for s in range(hi - lo):
                    t = lo + s
                    tg, tl = t >> 8, t & 255
                    st = t % 2
                    if t == 0:
                        for q in range(4):
                            nc.vector.tensor_copy(
                                arep[0:BC, 0, 16 * q:16 * (q + 1)],
                                pq[BC * q:BC * (q + 1), 0, :],
                            )
                    else:
                        x = wpool.tile([128, 16, T], f32, tag="x")
                        nc.vector._custom_dve(
                            segmax_op, out=x[:], in0=tsp[:], in1=arep_bc[1 - st]
                        )
                        # fused pot-add + collapse: alpha'[b, 16q+jb]
                        for q in range(4):
                            nc.vector.tensor_add(
                                arep[0:BC, st, 16 * q:16 * (q + 1)],
                                x[BC * q:BC * (q + 1), :, T - 1],
                                pq[BC * q:BC * (q + 1), s, :],
                            )
                    # rebroadcast alpha to the other 3 quadrant groups
                    nc.vector.tensor_copy(arep[BC:2 * BC, st, :], arep[0:BC, st, :])
                    nc.vector.tensor_copy(arep[2 * BC:4 * BC, st, :], arep[0:2 * BC, st, :])
                    # alpha history: one ACT copy per step-pair (slots 0,1 =
                    # alphas t-1, t when t is odd)
                    if st == 1:
                        nc.scalar.copy(
                            hist[BC * tg:BC * (tg + 1), tl - 1:tl + 1, :],
                            arep[0:BC, 0:2, :],
                        )

                    if tg < 3 and (t + 1) % 256 == 0:
                        nc.gpsimd.dma_start(
                            out=hist_out[:, 256 * tg:256 * (tg + 1), :],
                            in_=hist[BC * tg:BC * (tg + 1), :, :],
                        )
                    elif tg == 3 and (t + 1) % 64 == 0:
                        h0 = ((t + 1) // 64 - 1) * 64
                        nc.gpsimd.dma_start(
                            out=hist_out[:, h0:t + 1, :],
                            in_=hist[3 * BC:4 * BC, h0 - 768:t + 1 - 768, :],
                        )

    nc.compile()
    _cache["nc"] = nc
    return nc


def _make_tspread(trans):
    # tsp[32q + b, jb, i] = trans[i, 16q + jb]
    tt = np.ascontiguousarray(trans.T).reshape(4, 16, T)  # [q, jb, i]
    return np.repeat(tt[:, None, :, :], BC, axis=1).reshape(128, 16, T).astype(np.float32)


def _make_potq(pots_core):
    # potq[32q + b, t, jb] = pots_core[b, t, 16q + jb]
    return np.ascontiguousarray(
        pots_core.reshape(BC, L, 4, 16).transpose(2, 0, 1, 3).reshape(128, L, 16)
    )


def _make_in_maps(potentials, trans):
    tsp = _make_tspread(trans)
    return [
        {"potq": _make_potq(potentials[c * BC:(c + 1) * BC]), "tspread": tsp}
        for c in range(NCORES)
    ]


def kernel(potentials, lengths, transition_params):
    from concourse.bass_utils import run_bass_kernel_spmd

    potentials = np.ascontiguousarray(np.asarray(potentials, dtype=np.float32))
    lengths = np.asarray(lengths, dtype=np.int32)
    trans = np.ascontiguousarray(np.asarray(transition_params, dtype=np.float32))

    nc = _build_program()
    in_maps = _make_in_maps(potentials, trans)
    res = run_bass_kernel_spmd(nc, in_maps, core_ids=list(range(NCORES)))
    ah = np.concatenate([res.results[c]["ahist"] for c in range(NCORES)], axis=0)

    # Host backtrack over the device-computed alpha history.
    tags = np.zeros((B, L), dtype=np.int64)
    last = ah[np.arange(B), lengths - 1, :].argmax(axis=1)
    tags[:, L - 1] = last
    lm1 = lengths - 1
    for t in range(L - 2, -1, -1):
        nxt = tags[:, t + 1]
        cand = ah[:, t, :] + trans[:, nxt].T
        tags[:, t] = np.where(t >= lm1, last, cand.argmax(axis=1))
    return tags.astype(np.int32)


# revision 6
# speedup vs baseline: 1.0366x; 1.0045x over previous
"""Viterbi CRF decode on 8 Trainium2 NeuronCores — v2 (custom DVE segmax).

Data-parallel over batch (32 seqs/core). Per step, the max-plus inner loop
  m[b,j] = max_i fp32(alpha[b,i] + trans[i,j])
runs as ONE custom DVE instruction (VITERBI_SEGMAX): a fused
(Src0 + Src1) running-max scan with a page-boundary reset patched into the
uop program, over layout [128 partitions=(q,b), 16 pages=jb, 64=i] where
j = 16q + jb. Segment-end elements X[:, :, 63] are the 16 maxes per
partition, bitwise equal to the jax reference (single fp32 add, exact max).

The rest of the step stays on the DVE to avoid cross-engine sem latency:
4 fused "pot-add + collapse" TTs write alpha[b, :] directly into arep[0:32]
(cross-partition-offset outputs), then 2 doubling copies rebroadcast to all
128 partitions. The alpha history write goes to the Scalar engine
(off-critical-path). Host backtrack over the history as in v1.
"""

import numpy as np

B, L, T = 256, 1024, 64
NCORES = 8
BC = B // NCORES   # 32 sequences per core
CH = 128           # potq chunk (steps per input DMA)

_cache = {}


def _register_segmax():
    """Register the VITERBI_SEGMAX custom DVE op (idempotent).

    out[p,s,n] = running max within page s of fp32(in0[p,s,n] + in1[p,s,n]).
    The stock scan() has no page reset; we patch the lowering so the
    SUB_DIM_DONE step state re-seeds the scan stage from the current
    element's (Src0+Src1) value instead of folding the carried max.
    """
    import contextlib

    from concourse import dve_spec
    from concourse.dve_spec import AluOp, Spec, Src0, Src1, lower, scan
    from concourse.dve_uop import DveOpSpec
    from concourse.dve_ops import (
        _CUSTOM_DVE_ROW_BASE,
        _SUB_OPCODE_FOR_NAME,
        CUSTOM_DVE_SPECS,
        OPS,
        DveOp,
    )

    name = "VITERBI_SEGMAX"
    for op in OPS:
        if op.name == name:
            return op

    def _reference(in0, in1, **_kw):
        return np.maximum.accumulate(
            in0.astype(np.float32) + in1.astype(np.float32), axis=-1
        )

    spec = Spec(body=scan(AluOp.MAX, Src0 + Src1), reference=_reference)

    @contextlib.contextmanager
    def _page_reset_patch():
        orig = dve_spec._scan_overrides

        def patched(scans, node_stage):
            seed, step = orig(scans, node_stage)
            for s in scans:
                if s._subdim_step is None:
                    step[node_stage[s]] = dve_spec._Stage(AluOp.BYPASS, s.expr)
            return seed, step

        dve_spec._scan_overrides = patched
        try:
            yield
        finally:
            dve_spec._scan_overrides = orig

    row = _CUSTOM_DVE_ROW_BASE + len(OPS)
    _SUB_OPCODE_FOR_NAME[name] = row
    with _page_reset_patch():
        shas = {}
        for ver in ("v3", "v4"):
            uops = lower(spec, ver=ver)
            shas[ver] = DveOpSpec(
                name=name, opcode=row, uops=uops, rd1_en=True
            ).sha(ver)
        op = DveOp(name, spec, subdim=True, uops_sha=shas)
        OPS.append(op)
        CUSTOM_DVE_SPECS[name] = spec
        op.compile("v3")
        op.compile("v4")
    return op


def _build_program():
    if "nc" in _cache:
        return _cache["nc"]
    import concourse.bacc as bacc
    import concourse.mybir as mybir
    from concourse.tile import TileContext

    segmax_op = _register_segmax()

    f32 = mybir.dt.float32

    nc = bacc.Bacc("TRN2", target_bir_lowering=False, debug=False)
    potq_in = nc.dram_tensor("potq", [128, L, 16], f32, kind="ExternalInput").ap()
    tsp_in = nc.dram_tensor("tspread", [128, 16, T], f32, kind="ExternalInput").ap()
    hist_out = nc.dram_tensor("ahist", [BC, L, T], f32, kind="ExternalOutput").ap()

    with TileContext(nc) as tc:
        with tc.tile_pool(name="const", bufs=1) as cpool, \
             tc.tile_pool(name="pstream", bufs=2) as ppool, \
             tc.tile_pool(name="work", bufs=2) as wpool, \
             tc.tile_pool(name="big", bufs=1) as bpool:
            tsp = cpool.tile([128, 16, T], f32)
            nc.gpsimd.dma_start(out=tsp[:], in_=tsp_in[:])
            # two alpha slots (t%2): lets one ACT copy capture 2 steps of
            # history, halving the DVE<->ACT sem edges
            arep = cpool.tile([128, 4, T], f32)
            hist = bpool.tile([128, 256, T], f32)   # 64KB/partition

            arep_bc = [
                arep[:, k, :].unsqueeze(1).broadcast_to([128, 16, T])
                for k in range(4)
            ]

            CH0 = 16
            bounds = [0, CH0] + [CH0 + CH * k for k in range(1, (L - CH0) // CH + 1)]
            if bounds[-1] != L:
                bounds.append(L)
            for c in range(len(bounds) - 1):
                lo, hi = bounds[c], bounds[c + 1]
                pq = ppool.tile([128, CH, 16], f32, tag="potq")
                nc.gpsimd.dma_start(out=pq[:, 0:hi - lo, :], in_=potq_in[:, lo:hi, :])

                for s in range(hi - lo):
                    t = lo + s
                    tg, tl = t >> 8, t & 255
                    st = t % 4
                    if t == 0:
                        for q in range(4):
                            nc.vector.tensor_copy(
                                arep[0:BC, 0, 16 * q:16 * (q + 1)],
                                pq[BC * q:BC * (q + 1), 0, :],
                            )
                    else:
                        x = wpool.tile([128, 16, T], f32, tag="x")
                        nc.vector._custom_dve(
                            segmax_op, out=x[:], in0=tsp[:], in1=arep_bc[(t - 1) % 4]
                        )
                        # fused pot-add + collapse: alpha'[b, 16q+jb]
                        for q in range(4):
                            nc.vector.tensor_add(
                                arep[0:BC, st, 16 * q:16 * (q + 1)],
                                x[BC * q:BC * (q + 1), :, T - 1],
                                pq[BC * q:BC * (q + 1), s, :],
                            )
                    # rebroadcast alpha to the other 3 quadrant groups
                    nc.vector.tensor_copy(arep[BC:2 * BC, st, :], arep[0:BC, st, :])
                    nc.vector.tensor_copy(arep[2 * BC:4 * BC, st, :], arep[0:2 * BC, st, :])
                    # alpha history: one ACT copy per step-pair (slots 0,1 =
                    # alphas t-1, t when t is odd)
                    if st == 3:
                        nc.scalar.copy(
                            hist[BC * tg:BC * (tg + 1), tl - 3:tl + 1, :],
                            arep[0:BC, 0:4, :],
                        )

                    if tg < 3 and (t + 1) % 256 == 0:
                        nc.gpsimd.dma_start(
                            out=hist_out[:, 256 * tg:256 * (tg + 1), :],
                            in_=hist[BC * tg:BC * (tg + 1), :, :],
                        )
                    elif tg == 3 and (t + 1) % 64 == 0:
                        h0 = ((t + 1) // 64 - 1) * 64
                        nc.gpsimd.dma_start(
                            out=hist_out[:, h0:t + 1, :],
                            in_=hist[3 * BC:4 * BC, h0 - 768:t + 1 - 768, :],
                        )

    nc.compile()
    _cache["nc"] = nc
    return nc


def _make_tspread(trans):
    # tsp[32q + b, jb, i] = trans[i, 16q + jb]
    tt = np.ascontiguousarray(trans.T).reshape(4, 16, T)  # [q, jb, i]
    return np.repeat(tt[:, None, :, :], BC, axis=1).reshape(128, 16, T).astype(np.float32)


def _make_potq(pots_core):
    # potq[32q + b, t, jb] = pots_core[b, t, 16q + jb]
    return np.ascontiguousarray(
        pots_core.reshape(BC, L, 4, 16).transpose(2, 0, 1, 3).reshape(128, L, 16)
    )


def _make_in_maps(potentials, trans):
    tsp = _make_tspread(trans)
    return [
        {"potq": _make_potq(potentials[c * BC:(c + 1) * BC]), "tspread": tsp}
        for c in range(NCORES)
    ]


def kernel(potentials, lengths, transition_params):
    from concourse.bass_utils import run_bass_kernel_spmd

    potentials = np.ascontiguousarray(np.asarray(potentials, dtype=np.float32))
    lengths = np.asarray(lengths, dtype=np.int32)
    trans = np.ascontiguousarray(np.asarray(transition_params, dtype=np.float32))

    nc = _build_program()
    in_maps = _make_in_maps(potentials, trans)
    res = run_bass_kernel_spmd(nc, in_maps, core_ids=list(range(NCORES)))
    ah = np.concatenate([res.results[c]["ahist"] for c in range(NCORES)], axis=0)

    # Host backtrack over the device-computed alpha history.
    tags = np.zeros((B, L), dtype=np.int64)
    last = ah[np.arange(B), lengths - 1, :].argmax(axis=1)
    tags[:, L - 1] = last
    lm1 = lengths - 1
    for t in range(L - 2, -1, -1):
        nxt = tags[:, t + 1]
        cand = ah[:, t, :] + trans[:, nxt].T
        tags[:, t] = np.where(t >= lm1, last, cand.argmax(axis=1))
    return tags.astype(np.int32)


# revision 7
# speedup vs baseline: 1.0398x; 1.0031x over previous
"""Viterbi CRF decode on 8 Trainium2 NeuronCores — v2 (custom DVE segmax).

Data-parallel over batch (32 seqs/core). Per step, the max-plus inner loop
  m[b,j] = max_i fp32(alpha[b,i] + trans[i,j])
runs as ONE custom DVE instruction (VITERBI_SEGMAX): a fused
(Src0 + Src1) running-max scan with a page-boundary reset patched into the
uop program, over layout [128 partitions=(q,b), 16 pages=jb, 64=i] where
j = 16q + jb. Segment-end elements X[:, :, 63] are the 16 maxes per
partition, bitwise equal to the jax reference (single fp32 add, exact max).

The rest of the step stays on the DVE to avoid cross-engine sem latency:
4 fused "pot-add + collapse" TTs write alpha[b, :] directly into arep[0:32]
(cross-partition-offset outputs), then 2 doubling copies rebroadcast to all
128 partitions. The alpha history write goes to the Scalar engine
(off-critical-path). Host backtrack over the history as in v1.
"""

import numpy as np

B, L, T = 256, 1024, 64
NCORES = 8
BC = B // NCORES   # 32 sequences per core
CH = 128           # potq chunk (steps per input DMA)

_cache = {}


def _register_segmax():
    """Register the VITERBI_SEGMAX custom DVE op (idempotent).

    out[p,s,n] = running max within page s of fp32(in0[p,s,n] + in1[p,s,n]).
    The stock scan() has no page reset; we patch the lowering so the
    SUB_DIM_DONE step state re-seeds the scan stage from the current
    element's (Src0+Src1) value instead of folding the carried max.
    """
    import contextlib

    from concourse import dve_spec
    from concourse.dve_spec import AluOp, Spec, Src0, Src1, lower, scan
    from concourse.dve_uop import DveOpSpec
    from concourse.dve_ops import (
        _CUSTOM_DVE_ROW_BASE,
        _SUB_OPCODE_FOR_NAME,
        CUSTOM_DVE_SPECS,
        OPS,
        DveOp,
    )

    name = "VITERBI_SEGMAX"
    for op in OPS:
        if op.name == name:
            return op

    def _reference(in0, in1, **_kw):
        return np.maximum.accumulate(
            in0.astype(np.float32) + in1.astype(np.float32), axis=-1
        )

    spec = Spec(body=scan(AluOp.MAX, Src0 + Src1), reference=_reference)

    @contextlib.contextmanager
    def _page_reset_patch():
        orig = dve_spec._scan_overrides

        def patched(scans, node_stage):
            seed, step = orig(scans, node_stage)
            for s in scans:
                if s._subdim_step is None:
                    step[node_stage[s]] = dve_spec._Stage(AluOp.BYPASS, s.expr)
            return seed, step

        dve_spec._scan_overrides = patched
        try:
            yield
        finally:
            dve_spec._scan_overrides = orig

    row = _CUSTOM_DVE_ROW_BASE + len(OPS)
    _SUB_OPCODE_FOR_NAME[name] = row
    with _page_reset_patch():
        shas = {}
        for ver in ("v3", "v4"):
            uops = lower(spec, ver=ver)
            shas[ver] = DveOpSpec(
                name=name, opcode=row, uops=uops, rd1_en=True
            ).sha(ver)
        op = DveOp(name, spec, subdim=True, uops_sha=shas)
        OPS.append(op)
        CUSTOM_DVE_SPECS[name] = spec
        op.compile("v3")
        op.compile("v4")
    return op


def _build_program():
    if "nc" in _cache:
        return _cache["nc"]
    import concourse.bacc as bacc
    import concourse.mybir as mybir
    from concourse.tile import TileContext

    segmax_op = _register_segmax()

    f32 = mybir.dt.float32

    nc = bacc.Bacc("TRN2", target_bir_lowering=False, debug=False)
    potq_in = nc.dram_tensor("potq", [128, L, 16], f32, kind="ExternalInput").ap()
    tsp_in = nc.dram_tensor("tspread", [128, 16, T], f32, kind="ExternalInput").ap()
    hist_out = nc.dram_tensor("ahist", [BC, L, T], f32, kind="ExternalOutput").ap()

    with TileContext(nc) as tc:
        with tc.tile_pool(name="const", bufs=1) as cpool, \
             tc.tile_pool(name="pstream", bufs=2) as ppool, \
             tc.tile_pool(name="work", bufs=2) as wpool, \
             tc.tile_pool(name="big", bufs=1) as bpool:
            tsp = cpool.tile([128, 16, T], f32)
            nc.gpsimd.dma_start(out=tsp[:], in_=tsp_in[:])
            # two alpha slots (t%2): lets one ACT copy capture 2 steps of
            # history, halving the DVE<->ACT sem edges
            arep = cpool.tile([128, 8, T], f32)
            hist = bpool.tile([128, 256, T], f32)   # 64KB/partition

            arep_bc = [
                arep[:, k, :].unsqueeze(1).broadcast_to([128, 16, T])
                for k in range(8)
            ]

            CH0 = 16
            bounds = [0, CH0] + [CH0 + CH * k for k in range(1, (L - CH0) // CH + 1)]
            if bounds[-1] != L:
                bounds.append(L)
            for c in range(len(bounds) - 1):
                lo, hi = bounds[c], bounds[c + 1]
                pq = ppool.tile([128, CH, 16], f32, tag="potq")
                nc.gpsimd.dma_start(out=pq[:, 0:hi - lo, :], in_=potq_in[:, lo:hi, :])

                for s in range(hi - lo):
                    t = lo + s
                    tg, tl = t >> 8, t & 255
                    st = t % 8
                    if t == 0:
                        for q in range(4):
                            nc.vector.tensor_copy(
                                arep[0:BC, 0, 16 * q:16 * (q + 1)],
                                pq[BC * q:BC * (q + 1), 0, :],
                            )
                    else:
                        x = wpool.tile([128, 16, T], f32, tag="x")
                        nc.vector._custom_dve(
                            segmax_op, out=x[:], in0=tsp[:], in1=arep_bc[(t - 1) % 8]
                        )
                        # fused pot-add + collapse: alpha'[b, 16q+jb]
                        for q in range(4):
                            nc.vector.tensor_add(
                                arep[0:BC, st, 16 * q:16 * (q + 1)],
                                x[BC * q:BC * (q + 1), :, T - 1],
                                pq[BC * q:BC * (q + 1), s, :],
                            )
                    # rebroadcast alpha to the other 3 quadrant groups
                    nc.vector.tensor_copy(arep[BC:2 * BC, st, :], arep[0:BC, st, :])
                    nc.vector.tensor_copy(arep[2 * BC:4 * BC, st, :], arep[0:2 * BC, st, :])
                    # alpha history: one ACT copy per step-pair (slots 0,1 =
                    # alphas t-1, t when t is odd)
                    if st == 7:
                        nc.scalar.copy(
                            hist[BC * tg:BC * (tg + 1), tl - 7:tl + 1, :],
                            arep[0:BC, 0:8, :],
                        )

                    if tg < 3 and (t + 1) % 256 == 0:
                        nc.gpsimd.dma_start(
                            out=hist_out[:, 256 * tg:256 * (tg + 1), :],
                            in_=hist[BC * tg:BC * (tg + 1), :, :],
                        )
                    elif tg == 3 and (t + 1) % 64 == 0:
                        h0 = ((t + 1) // 64 - 1) * 64
                        nc.gpsimd.dma_start(
                            out=hist_out[:, h0:t + 1, :],
                            in_=hist[3 * BC:4 * BC, h0 - 768:t + 1 - 768, :],
                        )

    nc.compile()
    _cache["nc"] = nc
    return nc


def _make_tspread(trans):
    # tsp[32q + b, jb, i] = trans[i, 16q + jb]
    tt = np.ascontiguousarray(trans.T).reshape(4, 16, T)  # [q, jb, i]
    return np.repeat(tt[:, None, :, :], BC, axis=1).reshape(128, 16, T).astype(np.float32)


def _make_potq(pots_core):
    # potq[32q + b, t, jb] = pots_core[b, t, 16q + jb]
    return np.ascontiguousarray(
        pots_core.reshape(BC, L, 4, 16).transpose(2, 0, 1, 3).reshape(128, L, 16)
    )


def _make_in_maps(potentials, trans):
    tsp = _make_tspread(trans)
    return [
        {"potq": _make_potq(potentials[c * BC:(c + 1) * BC]), "tspread": tsp}
        for c in range(NCORES)
    ]


def kernel(potentials, lengths, transition_params):
    from concourse.bass_utils import run_bass_kernel_spmd

    potentials = np.ascontiguousarray(np.asarray(potentials, dtype=np.float32))
    lengths = np.asarray(lengths, dtype=np.int32)
    trans = np.ascontiguousarray(np.asarray(transition_params, dtype=np.float32))

    nc = _build_program()
    in_maps = _make_in_maps(potentials, trans)
    res = run_bass_kernel_spmd(nc, in_maps, core_ids=list(range(NCORES)))
    ah = np.concatenate([res.results[c]["ahist"] for c in range(NCORES)], axis=0)

    # Host backtrack over the device-computed alpha history.
    tags = np.zeros((B, L), dtype=np.int64)
    last = ah[np.arange(B), lengths - 1, :].argmax(axis=1)
    tags[:, L - 1] = last
    lm1 = lengths - 1
    for t in range(L - 2, -1, -1):
        nxt = tags[:, t + 1]
        cand = ah[:, t, :] + trans[:, nxt].T
        tags[:, t] = np.where(t >= lm1, last, cand.argmax(axis=1))
    return tags.astype(np.int32)
